# revision 26
# baseline (speedup 1.0000x reference)
"""Trainium2 Bass kernel for an 8-batch dense transformer block.

Reference computation (B=8, S=2048, E=1024, H=4096):
    Q = x@Wq + bq; K = x@Wk + bk; V = x@Wv + bv
    attn = softmax(mask(Q K^T) / sqrt(E))
    ctx  = attn @ LN1(V)
    h    = LN2(ctx)
    h    = relu(h@W1 + b1); h = relu(h@W2 + b2)
    out  = V + h

Strategy: pure data parallelism — one batch element per NeuronCore, weights
replicated, no collectives.  Host-side exact algebra folds:
  * scores = (x A) x^T with A = Wq Wk^T / sqrt(E)  (query/key row-bias terms
    are softmax-invariant; the key-column bias term is x (Wk bq)/sqrt(E),
    shipped separately when nonzero)
  * ln2_g/ln2_b folded into W1/b1
  * softmax denominator folded into the LN2 scalars (LN2 sees an exact
    rescale of the unnormalized attention output)
Matmuls run in bf16 (fp32 PSUM accumulation); norms/softmax in fp32.
"""

import os
import sys

if "/opt/trn_rl_repo" not in sys.path:
    sys.path.insert(0, "/opt/trn_rl_repo")

import numpy as np
import ml_dtypes

import concourse.bass as bass
import concourse.tile as tile
from concourse import mybir
from concourse.masks import make_identity

F32 = mybir.dt.float32
BF16 = mybir.dt.bfloat16

B, S, E, H = 8, 2048, 1024, 4096
SB = S // 128       # 16 token blocks
ET = E // 128       # 8 e tiles
HT = H // 128       # 32 h tiles
KC = S // 512       # 4 key chunks
EC = E // 512       # 2 feature chunks
EPS = 1e-5

LAST_EXEC_TIME_NS = None
LAST_RESULTS = None


# ---------------------------------------------------------------------------
# Workarounds: walrus here rejects >1 embedded sync-wait per instruction.
# ---------------------------------------------------------------------------
def _apply_patches():
    import bass_rust
    import concourse.tile as tile_mod
    from concourse.vector_clock import ScopedClock

    def _patched_drain_and_barrier(self, tick_clock, wait_clock):
        nc = self.nc
        drain_inst = nc.sync.drain()
        wait_clock.add_sem_waits(
            drain_inst.ins, ScopedClock({None: tick_clock.global_clock})
        )
        si = drain_inst.ins.sync_info
        waits = list(si.on_wait)
        drain_inst.ins.sync_info = bass_rust.SyncInfo(
            on_wait=[], on_update=list(si.on_update)
        )
        for w in waits:
            nop = nc.sync.nop(nofuse=True)
            nop.ins.sync_info = bass_rust.SyncInfo(on_wait=[w], on_update=[])
        nc.all_engine_barrier()
        assert self.sems is not None
        popped = nc._tile_sem_poison_stack.pop()
        assert popped is self._sem_poison
        nc.clear_and_free_semaphores(list(self.sems.allocated().values()))
        nc.all_engine_barrier()

    tile_mod.TileContext._drain_and_barrier = _patched_drain_and_barrier


def _fixup_waits(nc, max_waits=1):
    """Hoist excess embedded sync-waits onto NOPs preceding the instruction
    in its engine's program order."""
    import bass_rust

    n_fixed = 0
    for f in nc.m.functions:
        for bb in f.blocks:
            il = list(bb.instructions)
            out = []
            changed = False
            for inst in il:
                si = getattr(inst, "sync_info", None)
                waits = list(si.on_wait) if si is not None else []
                if len(waits) > max_waits:
                    keep = waits[:max_waits]
                    extra = waits[max_waits:]
                    for i, w in enumerate(extra):
                        nop = mybir.InstNoOp(
                            name=f"{inst.name}-waitfix-{i}",
                            sync_info=mybir.SyncInfo(on_wait=[w], on_update=[]),
                            bass_nofuse=True,
                            engine=inst.engine,
                        )
                        out.append(nop)
                    inst.sync_info = bass_rust.SyncInfo(
                        on_wait=keep, on_update=list(si.on_update)
                    )
                    changed = True
                    n_fixed += 1
                out.append(inst)
            if changed:
                bb.instructions = out
    return n_fixed


def _maybe_install_ntff_hook():
    """When tracing is requested, register the axon NTFF profile hook that
    the image's antenv lacks."""
    try:
        import types

        if "antenv.axon_hooks" in sys.modules:
            return
        from trn_agent_boot.trn_boot import _ntff_profile_via_ctypes

        hook = _ntff_profile_via_ctypes("/opt/axon/libaxon_pjrt.so")
        mod = types.ModuleType("antenv.axon_hooks")
        state = {"hook": hook}
        mod.set_axon_ntff_profile_hook = lambda h: state.__setitem__("hook", h)
        mod.get_axon_ntff_profile_hook = lambda: state["hook"]
        sys.modules["antenv.axon_hooks"] = mod
        import antenv

        antenv.axon_hooks = mod
    except Exception:
        pass


# ---------------------------------------------------------------------------
# Device graph
# ---------------------------------------------------------------------------
def _build(flags):
    """Build the per-core Bass graph. flags: has_colbias, has_vbias,
    has_ln1_affine, has_b2."""
    nc = bass.Bass(num_devices=8)

    xb = nc.declare_dram_parameter("xb", [E, S], BF16, isOutput=False)
    a_w = nc.declare_dram_parameter("a_w", [ET, 128, ET, 128], BF16, isOutput=False)
    wv_w = nc.declare_dram_parameter("wv_w", [128, ET, E], BF16, isOutput=False)
    w1_w = nc.declare_dram_parameter("w1_w", [HT, 128, ET, 128], BF16, isOutput=False)
    w2_w = nc.declare_dram_parameter("w2_w", [H, E], BF16, isOutput=False)
    b1_w = nc.declare_dram_parameter("b1_w", [128, HT], F32, isOutput=False)
    if flags["has_colbias"]:
        cb_w = nc.declare_dram_parameter("cb_w", [1, S], F32, isOutput=False)
    if flags["has_vbias"]:
        bv_w = nc.declare_dram_parameter("bv_w", [1, E], F32, isOutput=False)
    if flags["has_ln1_affine"]:
        g1_w = nc.declare_dram_parameter("g1_w", [1, E], F32, isOutput=False)
        c1_w = nc.declare_dram_parameter("c1_w", [1, E], F32, isOutput=False)
    if flags["has_b2"]:
        b2_w = nc.declare_dram_parameter("b2_w", [1, E], F32, isOutput=False)
    out_w = nc.declare_dram_parameter("out", [S, E], F32, isOutput=True)

    vscr = nc.dram_tensor("vscr", [SB, 128, E], F32)

    Exp = mybir.ActivationFunctionType.Exp
    Relu = mybir.ActivationFunctionType.Relu
    Sqrt = mybir.ActivationFunctionType.Sqrt
    Ln = mybir.ActivationFunctionType.Ln
    AX = mybir.AxisListType.X
    MAX = mybir.AluOpType.max
    SUB = mybir.AluOpType.subtract
    MUL = mybir.AluOpType.mult

    with tile.TileContext(nc) as tc:
        import contextlib

        stack = contextlib.ExitStack()
        with stack:
            const = stack.enter_context(tc.tile_pool(name="const", bufs=1))
            ident = const.tile([128, 128], BF16)
            make_identity(nc, ident[:])
            eps_t = const.tile([128, 1], F32)
            nc.vector.memset(eps_t[:], EPS)
            negC = const.tile([128, 1], F32)
            nc.vector.memset(negC[:], -20.0)
            b1_sb = const.tile([128, HT], F32)
            nc.sync.dma_start(b1_sb[:], b1_w[:])
            if flags["has_colbias"]:
                cb_sb = const.tile([128, S], F32)
                nc.sync.dma_start(cb_sb[:], cb_w[:].broadcast_to([128, S]))
            if flags["has_vbias"]:
                bv_sb = const.tile([128, E], F32)
                nc.sync.dma_start(bv_sb[:], bv_w[:].broadcast_to([128, E]))
            if flags["has_ln1_affine"]:
                g1_sb = const.tile([128, E], F32)
                nc.sync.dma_start(g1_sb[:], g1_w[:].broadcast_to([128, E]))
                c1_sb = const.tile([128, E], F32)
                nc.sync.dma_start(c1_sb[:], c1_w[:].broadcast_to([128, E]))
            if flags["has_b2"]:
                b2_sb = const.tile([128, E], F32)
                nc.sync.dma_start(b2_sb[:], b2_w[:].broadcast_to([128, E]))

            # Long-lived activations. Stack order matters: hT lives through
            # FFN1; vn/xT/qT are released after phase 2 so the FFN phase can
            # reuse their SBUF.
            acts_ht = stack.enter_context(tc.tile_pool(name="acts_ht", bufs=1))
            hT = acts_ht.tile([128, ET, S], BF16)   # LN2(ctx)^T (feature-major)
            w1boot = stack.enter_context(tc.tile_pool(name="w1boot", bufs=1))
            w1b_sb = w1boot.tile([128, 6, ET, 128], BF16)
            for hb in range(6):
                nc.sync.dma_start(w1b_sb[:, hb, :, :], w1_w[hb])
            h1boot_pool = stack.enter_context(tc.tile_pool(name="h1boot", bufs=1))
            h1T_boot = h1boot_pool.tile([128, HT, 512], BF16)
            psH_pool = stack.enter_context(
                tc.tile_pool(name="psH", bufs=1, space="PSUM")
            )
            acts_vn_cm = tc.tile_pool(name="acts_vn", bufs=1)
            acts_vn = acts_vn_cm.__enter__()
            vn = acts_vn.tile([128, SB, E], BF16)   # LN1(V) (token-major)
            acts_xq_cm = tc.tile_pool(name="acts_xq", bufs=1)
            acts_xq = acts_xq_cm.__enter__()
            xT = acts_xq.tile([128, ET, S], BF16)   # x^T  (feature-major)
            qT = acts_xq.tile([128, ET, S], BF16)   # (xA)^T

            # ---------------- phase 0: load pre-transposed x --------------
            _dma_engines = [nc.sync, nc.gpsimd, nc.scalar]
            for et in range(ET):
                _dma_engines[et % 3].dma_start(
                    xT[:, et, :], xb[et * 128 : (et + 1) * 128, :]
                )

            # ---------------- phase 1: q' = xA (transposed), V + LN1 ------
            with tc.tile_pool(name="p1sb", bufs=1) as p1sb, \
                 tc.tile_pool(name="p1a", bufs=4) as p1a, \
                 tc.tile_pool(name="p1v", bufs=2) as p1v, \
                 tc.tile_pool(name="p1small", bufs=4) as p1small, \
                 tc.tile_pool(name="p1ps", bufs=3, space="PSUM") as p1ps, \
                 tc.tile_pool(name="p1psv", bufs=3, space="PSUM") as p1psv:
                wv_sb = p1sb.tile([128, ET, E], BF16)
                for et in range(ET):
                    _dma_engines[(et + 1) % 3].dma_start(wv_sb[:, et, :], wv_w[:, et, :])

                # q'^T[f, s] — accumulate over e tiles
                for fb in range(ET):
                    a_sb = p1a.tile([128, ET, 128], BF16)
                    _dma_engines[fb % 3].dma_start(a_sb[:], a_w[fb])
                    for sc in range(KC):
                        ps_q = p1ps.tile([128, 512], F32)
                        for et in range(ET):
                            nc.tensor.matmul(
                                ps_q[:],
                                a_sb[:, et, :],
                                xT[:, et, sc * 512 : (sc + 1) * 512],
                                start=(et == 0),
                                stop=(et == ET - 1),
                            )
                        nc.scalar.copy(qT[:, fb, sc * 512 : (sc + 1) * 512], ps_q[:])

                # V[s, f] token-major; LN1 fused on evacuation
                for si in range(SB):
                    ps_v = []
                    for fc in range(EC):
                        pv = p1psv.tile([128, 512], F32)
                        ps_v.append(pv)
                        for et in range(ET):
                            nc.tensor.matmul(
                                pv[:],
                                xT[:, et, si * 128 : (si + 1) * 128],
                                wv_sb[:, et, fc * 512 : (fc + 1) * 512],
                                start=(et == 0),
                                stop=(et == ET - 1),
                            )
                    v_sb = p1v.tile([128, E], F32)
                    for fc in range(EC):
                        nc.scalar.copy(v_sb[:, fc * 512 : (fc + 1) * 512], ps_v[fc][:])
                    if flags["has_vbias"]:
                        nc.vector.tensor_add(v_sb[:], v_sb[:], bv_sb[:])
                    # LN1 stats
                    st = p1small.tile([128, EC, 6], F32)
                    for fc in range(EC):
                        nc.vector.bn_stats(st[:, fc, :], v_sb[:, fc * 512 : (fc + 1) * 512])
                    mv = p1small.tile([128, 2], F32)
                    nc.vector.bn_aggr(mv[:], st[:])
                    lnv = p1small.tile([128, 1], F32)
                    nc.scalar.activation(lnv[:], mv[:, 1:2], Ln, bias=eps_t[:])
                    rstd = p1small.tile([128, 1], F32)
                    nc.scalar.activation(rstd[:], lnv[:], Exp, scale=-0.5)
                    nc.vector.tensor_scalar(
                        out=vn[:, si, :], in0=v_sb[:], scalar1=mv[:, 0:1],
                        scalar2=rstd[:], op0=SUB, op1=MUL,
                    )
                    if flags["has_ln1_affine"]:
                        nc.vector.tensor_mul(vn[:, si, :], vn[:, si, :], g1_sb[:])
                        nc.vector.tensor_add(vn[:, si, :], vn[:, si, :], c1_sb[:])
                    nc.sync.dma_start(vscr[si], v_sb[:])

            # ---------------- phase 2: attention + LN2 ----------------
            with tc.tile_pool(name="p2p", bufs=2) as p2p, \
                 tc.tile_pool(name="p2small", bufs=6) as p2small, \
                 tc.tile_pool(name="p2h", bufs=2) as p2h, \
                 tc.tile_pool(name="psS", bufs=4, space="PSUM") as psS_pool, \
                 tc.tile_pool(name="psT", bufs=1, space="PSUM") as psT_pool, \
                 tc.tile_pool(name="psC", bufs=2, space="PSUM") as psC_pool:
                for qi in range(SB):
                    # scores (pre-scaled by 1/sqrt(E) via A). Softmax uses a
                    # fixed offset C (shift-invariance): scores here are
                    # O(1)-scaled, so exp(s - C) stays in fp32/bf16 range.
                    p_bf = p2p.tile([128, S], BF16, tag="p")
                    lacc = p2small.tile([128, KC], F32)
                    for kc in range(KC):
                        ps = psS_pool.tile([128, 512], F32, tag="scores")
                        for et in range(ET):
                            nc.tensor.matmul(
                                ps[:],
                                qT[:, et, qi * 128 : (qi + 1) * 128],
                                xT[:, et, kc * 512 : (kc + 1) * 512],
                                start=(et == 0),
                                stop=(et == ET - 1),
                            )
                        if flags["has_colbias"]:
                            nc.vector.tensor_add(
                                ps[:], ps[:], cb_sb[:, kc * 512 : (kc + 1) * 512]
                            )
                        nc.scalar.activation(
                            p_bf[:, kc * 512 : (kc + 1) * 512], ps[:], Exp,
                            bias=negC[:], accum_out=lacc[:, kc : kc + 1],
                        )
                    lsum = p2small.tile([128, 1], F32)
                    nc.vector.tensor_reduce(
                        lsum[:], lacc[:], axis=AX, op=mybir.AluOpType.add
                    )
                    # transpose P
                    pT = p2p.tile([128, SB, 128], BF16, tag="pT")
                    for g in range(4):
                        ps_t = psT_pool.tile([128, 512], BF16, tag="pstr", name="ps_t")
                        for j in range(4):
                            kt = 4 * g + j
                            nc.tensor.transpose(
                                ps_t[:, j * 128 : (j + 1) * 128],
                                p_bf[:, kt * 128 : (kt + 1) * 128],
                                ident[:],
                            )
                        nc.vector.tensor_copy(
                            pT[:, 4 * g : 4 * g + 4, :],
                            ps_t[:].rearrange("p (a b) -> p a b", a=4),
                        )
                    # ctx = P~ @ Vn (unnormalized)
                    ps_c = []
                    for ec in range(EC):
                        pc = psC_pool.tile([128, 512], F32)
                        ps_c.append(pc)
                        for kt in range(SB):
                            nc.tensor.matmul(
                                pc[:],
                                pT[:, kt, :],
                                vn[:, kt, ec * 512 : (ec + 1) * 512],
                                start=(kt == 0),
                                stop=(kt == SB - 1),
                            )
                    # LN2 with softmax normalization folded in (exact):
                    # c = u/l;  h = (u - mu_u) * (rstd_c / l),
                    # rstd_c = 1/sqrt(var_u/l^2 + eps)
                    st2 = p2small.tile([128, EC, 6], F32, tag="st2")
                    for ec in range(EC):
                        nc.vector.bn_stats(st2[:, ec, :], ps_c[ec][:])
                    mv2 = p2small.tile([128, 2], F32, tag="mv2")
                    nc.vector.bn_aggr(mv2[:], st2[:])
                    sinv = p2small.tile([128, 1], F32, tag="sinv")
                    nc.vector.reciprocal(sinv[:], lsum[:])
                    t1 = p2small.tile([128, 1], F32, tag="t1")
                    nc.vector.tensor_mul(t1[:], mv2[:, 1:2], sinv[:])
                    nc.vector.tensor_mul(t1[:], t1[:], sinv[:])
                    lnv2 = p2small.tile([128, 1], F32, tag="lnv2")
                    nc.scalar.activation(lnv2[:], t1[:], Ln, bias=eps_t[:])
                    rstd2 = p2small.tile([128, 1], F32, tag="rstd2")
                    nc.scalar.activation(rstd2[:], lnv2[:], Exp, scale=-0.5)
                    fac = p2small.tile([128, 1], F32, tag="fac")
                    nc.vector.tensor_mul(fac[:], rstd2[:], sinv[:])
                    h_tok = p2h.tile([128, E], BF16)
                    for ec in range(EC):
                        nc.vector.tensor_scalar(
                            out=h_tok[:, ec * 512 : (ec + 1) * 512], in0=ps_c[ec][:],
                            scalar1=mv2[:, 0:1], scalar2=fac[:], op0=SUB, op1=MUL,
                        )
                    # transpose h into hT
                    for g in range(2):
                        ps_t2 = psT_pool.tile([128, 512], BF16, tag="pstr", name="ps_t2")
                        for j in range(4):
                            fb = 4 * g + j
                            nc.tensor.transpose(
                                ps_t2[:, j * 128 : (j + 1) * 128],
                                h_tok[:, fb * 128 : (fb + 1) * 128],
                                ident[:],
                            )
                        nc.vector.tensor_copy(
                            hT[:, 4 * g : 4 * g + 4, qi * 128 : (qi + 1) * 128],
                            ps_t2[:].rearrange("p (a b) -> p a b", a=4),
                        )

            acts_xq_cm.__exit__(None, None, None)
            acts_vn_cm.__exit__(None, None, None)

            # ---------------- phase 3: FFN + residual ----------------
            with tc.tile_pool(name="p3h1", bufs=1) as p3h1, \
                 tc.tile_pool(name="p3w1", bufs=8) as p3w1, \
                 tc.tile_pool(name="p3w2", bufs=1) as p3w2, \
                 tc.tile_pool(name="p3o", bufs=3) as p3o, \
                 tc.tile_pool(name="p3v", bufs=3) as p3v, \
                 tc.tile_pool(name="psO", bufs=6, space="PSUM") as psO_pool:
                w2_sb = p3w2.tile([128, HT, E], BF16)
                for ht in range(HT):
                    nc.sync.dma_start(
                        w2_sb[:, ht, :], w2_w[ht * 128 : (ht + 1) * 128, :]
                    )
                for sc in range(KC):  # 4 chunks of 512 tokens
                    if sc == 0:
                        h1T = h1T_boot
                    else:
                        h1T = p3h1.tile([128, HT, 512], BF16, tag="h1T")
                    for hb in range(HT):
                        if sc == 0 and hb < 6:
                            w1_slice = w1b_sb[:, hb, :, :]
                        else:
                            w1_sb = p3w1.tile([128, ET, 128], BF16)
                            nc.sync.dma_start(w1_sb[:], w1_w[hb])
                            w1_slice = w1_sb[:]
                        ps_h = psH_pool.tile([128, 512], F32)
                        for et in range(ET):
                            nc.tensor.matmul(
                                ps_h[:],
                                w1_slice[:, et, :],
                                hT[:, et, sc * 512 : (sc + 1) * 512],
                                start=(et == 0),
                                stop=(et == ET - 1),
                            )
                        nc.scalar.activation(
                            h1T[:, hb, :], ps_h[:], Relu, bias=b1_sb[:, hb : hb + 1]
                        )
                    # second FFN layer + residual for the 4 s-blocks in chunk
                    for ec in range(EC):
                        ps_o = [
                            psO_pool.tile([128, 512], F32, tag="pso", name="pso") for _ in range(4)
                        ]
                        for ht in range(HT):
                            for j in range(4):
                                nc.tensor.matmul(
                                    ps_o[j][:],
                                    h1T[:, ht, j * 128 : (j + 1) * 128],
                                    w2_sb[:, ht, ec * 512 : (ec + 1) * 512],
                                    start=(ht == 0),
                                    stop=(ht == HT - 1),
                                )
                        for j in range(4):
                            si = sc * 4 + j
                            if flags["has_b2"]:
                                nc.vector.tensor_add(
                                    ps_o[j][:], ps_o[j][:],
                                    b2_sb[:, ec * 512 : (ec + 1) * 512],
                                )
                            o_sb = p3o.tile([128, 512], F32)
                            nc.scalar.activation(o_sb[:], ps_o[j][:], Relu)
                            v_ld = p3v.tile([128, 512], F32)
                            nc.sync.dma_start(
                                v_ld[:], vscr[si, :, ec * 512 : (ec + 1) * 512]
                            )
                            nc.vector.tensor_add(o_sb[:], o_sb[:], v_ld[:])
                            nc.sync.dma_start(
                                out_w[si * 128 : (si + 1) * 128, ec * 512 : (ec + 1) * 512],
                                o_sb[:],
                            )

    _fixup_waits(nc)
    return nc


# ---------------------------------------------------------------------------
# Host wrapper
# ---------------------------------------------------------------------------
def kernel(
    xembeddings, mask, Wq_w, Wq_b, Wk_w, Wk_b, Wv_w, Wv_b,
    ln1_g, ln1_b, ln2_g, ln2_b, W1, b1, W2, b2,
):
    global LAST_EXEC_TIME_NS, LAST_RESULTS
    _apply_patches()
    trace = bool(os.environ.get("BASS_TRACE"))
    if trace:
        _maybe_install_ntff_hook()

    x = np.asarray(xembeddings, dtype=np.float32)
    mask = np.asarray(mask)
    f64 = np.float64

    # host-side exact folds (float64)
    A = (np.asarray(Wq_w, f64) @ np.asarray(Wk_w, f64).T) / np.sqrt(E)
    W1f = np.asarray(ln2_g, f64)[:, None] * np.asarray(W1, f64)
    b1f = np.asarray(b1, f64) + np.asarray(ln2_b, f64) @ np.asarray(W1, f64)

    # column bias on scores from the query bias: (x @ (Wk @ bq)) / sqrt(E)
    colbias = (x.astype(f64) @ (np.asarray(Wk_w, f64) @ np.asarray(Wq_b, f64))) / np.sqrt(E)
    maskbias = np.where(np.asarray(mask, bool), 0.0, -1e30)  # [B, S]
    cb = colbias + maskbias  # [B, S]
    has_colbias = bool(np.any(cb != 0.0))

    bv = np.asarray(Wv_b, np.float32)
    has_vbias = bool(np.any(bv != 0.0))
    g1 = np.asarray(ln1_g, np.float32)
    c1 = np.asarray(ln1_b, np.float32)
    has_ln1_affine = bool(np.any(g1 != 1.0) or np.any(c1 != 0.0))
    b2f = np.asarray(b2, np.float32)
    has_b2 = bool(np.any(b2f != 0.0))

    flags = {
        "has_colbias": has_colbias,
        "has_vbias": has_vbias,
        "has_ln1_affine": has_ln1_affine,
        "has_b2": has_b2,
    }

    bf = ml_dtypes.bfloat16
    # weight layouts (see _build):
    #   a_w/wv_w: [128 e_p, ET, E_out]  (per-partition contiguous)
    a_h = (A.astype(np.float32).astype(bf).reshape(ET, 128, ET, 128).transpose(2, 1, 0, 3).copy())
    wv_h = (
        np.asarray(Wv_w, np.float32).astype(bf).reshape(ET, 128, E).transpose(1, 0, 2).copy()
    )
    #   w1_w: [HT, 128 e_p, ET, 128 f]
    w1_h = (
        W1f.astype(np.float32).astype(bf)
        .reshape(ET, 128, HT, 128).transpose(2, 1, 0, 3).copy()
    )
    w2_h = np.asarray(W2, np.float32).astype(bf).copy()
    b1_h = b1f.astype(np.float32).reshape(HT, 128).T.copy()

    nc = _build(flags)

    in_maps = []
    for b_i in range(B):
        m = {
            "xb": np.ascontiguousarray(x[b_i].T).astype(bf),
            "a_w": a_h,
            "wv_w": wv_h,
            "w1_w": w1_h,
            "w2_w": w2_h,
            "b1_w": b1_h,
        }
        if has_colbias:
            m["cb_w"] = cb[b_i].astype(np.float32).reshape(1, S)
        if has_vbias:
            m["bv_w"] = bv.reshape(1, E)
        if has_ln1_affine:
            m["g1_w"] = g1.reshape(1, E)
            m["c1_w"] = c1.reshape(1, E)
        if has_b2:
            m["b2_w"] = b2f.reshape(1, E)
        in_maps.append(m)

    from concourse.bass_utils import run_bass_kernel_spmd

    res = run_bass_kernel_spmd(
        nc, in_maps, core_ids=list(range(B)), trace=trace
    )
    LAST_EXEC_TIME_NS = res.exec_time_ns
    LAST_RESULTS = res
    out = np.stack([res.results[i]["out"] for i in range(B)], axis=0)
    return out.astype(np.float32)
